# revision 55
# baseline (speedup 1.0000x reference)
"""Trainium2 Bass kernel for nn_JointLearningModel (coref-style joint model).

Sharding: the 384x384 pair grid is split by rows across 8 NeuronCores.
Mention representations are computed on the host (pure gathers) and
uploaded replicated in transposed [H, N] layout; params replicated; the
scalar loss is computed per-core over its row block (+ its slice of the
character CE) and summed on the host.

Key optimizations over the naive version:
- The causal mask means row i only needs pair columns j < i. Rows are
  dealt to cores by column-chunk class (1, 2 or 3 x128 chunks) so all
  cores run the identical SPMD instruction stream but skip ~1/3 of the
  pair-MLP work.
- The dominant W_pair2 matmul (and the W_pair3 reduction) run in fp8
  (e4m3, weights pre-scaled by 16) with DoubleRow perf mode: 2x PE
  throughput, contraction 256 per instruction.
- Score rows move PSUM->SBUF via DMA instead of vector/scalar copies.
"""

import numpy as np
import ml_dtypes

import concourse.bass as bass
import concourse.mybir as mybir
import concourse.tile as tile
from concourse import bacc
from concourse.bass_utils import run_bass_kernel_spmd

F32 = mybir.dt.float32
BF16 = mybir.dt.bfloat16
F8 = mybir.dt.float8e4
I32 = mybir.dt.int32
AF = mybir.ActivationFunctionType
OP = mybir.AluOpType
DR = mybir.MatmulPerfMode.DoubleRow

B, L, H, M = 8, 512, 768, 383
N = M + 1          # 384 rows/cols of the pair grid
NC_ = 8            # cores
R = N // NC_       # 48 rows per core
HC = H // 128      # 6 k-chunks of the hidden dim
NEG = -10000.0
S2 = 16.0          # fp8 pre-scale on W_pair2
S3 = 16.0          # fp8 pre-scale on W_pair3

_CACHE = {}
LAST_RESULT = None


def _build_program(
    reps=1, fuse_relu=True, h1_gp=0, h1_bufs=8, h2t_bufs=4, copy_mode="alt"
):
    nc = bacc.Bacc(
        "TRN2", target_bir_lowering=False, debug=False, enable_asserts=False
    )

    def din(name, shape, dt):
        return nc.dram_tensor(name, list(shape), dt, kind="ExternalInput")

    # mention representations (host-gathered), transposed layouts
    reps8 = din("reps8", [128, HC, N], F8)      # reps8[p,c,j] = reps[j, 128c+p]
    repsTl = din("repsTl", [128, HC, R], BF16)  # local rows, slot order
    repsTl8 = din("repsTl8", [128, HC, R], F8)
    # pair MLP weights (fp8, pre-scaled by S2)
    wa8 = din("wa8", [128, HC, H], F8)          # wa8[p,ci,o] = Wa[o, 128ci+p]*S2
    wb8 = din("wb8", [128, HC, H], F8)
    w28 = din("w28", [128, HC, H // 2], F8)     # W2.T * S2, fp8
    # inner dim padded to 16 so the DoubleRow k-pair stride is 16B-aligned
    w38 = din("w38", [128, 3, 16], F8)          # W3 * S3 in [:, :, 0], fp8
    b1c = din("b1c", [128, HC], F32)
    b2c = din("b2c", [128, 3], F32)
    # mention-score MLP
    wm18 = din("wm18", [128, HC, H // 2], F8)   # W_m1.T * S2, fp8
    bm1c = din("bm1c", [128, 3], F32)
    wm2T = din("wm2T", [128, 3, H // 4], BF16)
    bm2c = din("bm2c", [128, 2], F32)
    wm3c = din("wm3c", [128, 2], BF16)
    # character head
    wc1T = din("wc1T", [128, HC, H // 2], BF16)
    bc1c = din("bc1c", [128, 3], F32)
    wc2T = din("wc2T", [128, 3, 18], BF16)
    bc2r = din("bc2r", [1, 18], F32)
    # per-core loss plumbing (rows in slot order)
    maskb = din("maskb", [R, N], F32)
    multb = din("multb", [R, N], F32)
    wnll = din("wnll", [R, 1], F32)
    oneh = din("oneh", [R, 18], F32)
    wch = din("wch", [R, 1], F32)
    zrow = din("zrow", [R, N], F32)             # zeros, pre-clears sblk

    loss = nc.dram_tensor("loss", [1, 1], F32, kind="ExternalOutput")

    opts = dict(fuse_relu=fuse_relu, h1_gp=h1_gp, h1_bufs=h1_bufs,
                h2t_bufs=h2t_bufs, copy_mode=copy_mode)
    with tile.TileContext(nc) as tc:
        for rep in range(reps):
            _build_rep(nc, tc, rep, opts, dict(
                reps8=reps8, repsTl=repsTl, repsTl8=repsTl8, wa8=wa8,
                wb8=wb8, w28=w28, w38=w38, b1c=b1c, b2c=b2c, wm18=wm18,
                bm1c=bm1c, wm2T=wm2T, bm2c=bm2c, wm3c=wm3c, wc1T=wc1T,
                bc1c=bc1c, wc2T=wc2T, bc2r=bc2r, maskb=maskb,
                multb=multb, wnll=wnll, oneh=oneh, wch=wch, zrow=zrow,
                loss=loss,
            ))

    nc.compile()
    return nc


def _build_rep(nc, tc, rep, opts, io):
    fuse_relu = opts["fuse_relu"]
    with tc.tile_pool(name=f"const{rep}", bufs=1) as cp:
        def load(name, h, q=None):
            t = cp.tile(list(h.shape), h.dtype, name=f"{name}{rep}")
            (q or nc.sync).dma_start(out=t[:], in_=h.ap())
            return t

        # split the loads over both HWDGE queues (SP + ACT) so the A.T
        # and Bb input chains start in parallel
        reps8_sb = load("reps8_sb", io["reps8"])
        wa8_sb = load("wa8_sb", io["wa8"])
        wb8_sb = load("wb8_sb", io["wb8"], q=nc.scalar)
        repsTl_sb = load("repsTl_sb", io["repsTl"])
        repsTl8_sb = load("repsTl8_sb", io["repsTl8"], q=nc.scalar)
        w28_sb = load("w28_sb", io["w28"])
        w38_sb = load("w38_sb", io["w38"])
        b1c_sb = load("b1c_sb", io["b1c"], q=nc.scalar)
        b2c_sb = None if fuse_relu else load("b2c_sb", io["b2c"])
        wm18_sb = load("wm18_sb", io["wm18"])
        bm1c_sb = load("bm1c_sb", io["bm1c"])
        wm2T_sb = load("wm2T_sb", io["wm2T"])
        bm2c_sb = load("bm2c_sb", io["bm2c"])
        wm3c_sb = load("wm3c_sb", io["wm3c"])
        wc1T_sb = load("wc1T_sb", io["wc1T"], q=nc.scalar)
        bc1c_sb = load("bc1c_sb", io["bc1c"], q=nc.scalar)
        wc2T_sb = load("wc2T_sb", io["wc2T"], q=nc.scalar)
        bc2r_sb = load("bc2r_sb", io["bc2r"], q=nc.scalar)
        maskb_sb = load("maskb_sb", io["maskb"], q=nc.scalar)
        multb_sb = load("multb_sb", io["multb"], q=nc.scalar)
        wnll_sb = load("wnll_sb", io["wnll"], q=nc.scalar)
        oneh_sb = load("oneh_sb", io["oneh"], q=nc.scalar)
        wch_sb = load("wch_sb", io["wch"], q=nc.scalar)

        one1 = cp.tile([1, R], F32, name=f"one1{rep}")
        nc.vector.memset(one1[:], 1.0)

        # outputs of the preamble, used by the main loop / epilogue
        at_sb = cp.tile([128, HC, N], BF16, name=f"at_sb{rep}")   # A.T
        bb_sb = cp.tile([128, HC, R], F32, name=f"bb_sb{rep}")    # Bm.T + b1
        mskms = cp.tile([R, N], F32, name=f"mskms{rep}")          # mask+ms[j]
        sblkf = cp.tile([1, R, N], F32, name=f"sblkf{rep}")       # scores flat
        # Destination of the reshape; pre-zeroed (fast: spreads over 48
        # partitions) so rows' uncomputed masked tails read as 0.
        sblk = cp.tile([R, N], F32, name=f"sblk{rep}")
        nc.sync.dma_start(out=sblk[:], in_=io["zrow"].ap())

        # ---------- preamble matmuls: A.T, Bb, ms, mask+ms ----------
        with tc.tile_pool(name=f"pre_ps{rep}", bufs=2, space="PSUM") as pp:
            for co in range(HC):
                pa = pp.tile([128, N], F32, tag="big", name=f"pa{rep}_{co}")
                for ci in range(0, HC, 2):
                    nc.tensor.matmul(
                        out=pa[:],
                        lhsT=wa8_sb[:, ci : ci + 2, co * 128 : (co + 1) * 128],
                        rhs=reps8_sb[:, ci : ci + 2, :],
                        start=(ci == 0),
                        stop=(ci == HC - 2),
                        perf_mode=DR,
                    )
                nc.scalar.mul(out=at_sb[:, co, :], in_=pa[:], mul=1.0 / S2)
            for co in range(HC):
                pb = pp.tile([128, R], F32, tag="small", name=f"pb{rep}_{co}")
                for ci in range(HC):
                    nc.tensor.matmul(
                        out=pb[:],
                        lhsT=wb8_sb[:, ci, co * 128 : (co + 1) * 128],
                        rhs=repsTl8_sb[:, ci, :],
                        start=(ci == 0),
                        stop=(ci == HC - 1),
                    )
                nc.vector.tensor_scalar(
                    out=bb_sb[:, co, :],
                    in0=pb[:],
                    scalar1=1.0 / S2,
                    scalar2=b1c_sb[:, co : co + 1],
                    op0=OP.mult,
                    op1=OP.add,
                )
            # mention score MLP (768 -> 384 -> 192 -> 1)
            ms1 = cp.tile([128, 3, N], BF16, name=f"ms1{rep}")
            for co in range(3):
                pm = pp.tile([128, N], F32, tag="big", name=f"pm{rep}_{co}")
                for ci in range(0, HC, 2):
                    nc.tensor.matmul(
                        out=pm[:],
                        lhsT=wm18_sb[:, ci : ci + 2, co * 128 : (co + 1) * 128],
                        rhs=reps8_sb[:, ci : ci + 2, :],
                        start=(ci == 0),
                        stop=(ci == HC - 2),
                        perf_mode=DR,
                    )
                nc.scalar.activation(
                    out=ms1[:, co, :],
                    in_=pm[:],
                    func=AF.Relu,
                    bias=bm1c_sb[:, co : co + 1],
                    scale=1.0 / S2,
                )
            ms2 = cp.tile([128, 2, N], BF16, name=f"ms2{rep}")
            for co, sz in enumerate((128, 64)):
                pm2 = pp.tile([128, N], F32, tag="big", name=f"pm2{rep}_{co}")
                for ci in range(3):
                    nc.tensor.matmul(
                        out=pm2[:sz, :],
                        lhsT=wm2T_sb[:, ci, co * 128 : co * 128 + sz],
                        rhs=ms1[:, ci, :],
                        start=(ci == 0),
                        stop=(ci == 2),
                    )
                nc.scalar.activation(
                    out=ms2[:sz, co, :],
                    in_=pm2[:sz, :],
                    func=AF.Relu,
                    bias=bm2c_sb[:sz, co : co + 1],
                )
            pms = pp.tile([1, N], F32, tag="small", name=f"pms{rep}")
            nc.tensor.matmul(
                out=pms[:], lhsT=wm3c_sb[:, 0:1], rhs=ms2[:, 0, :],
                start=True, stop=False,
            )
            nc.tensor.matmul(
                out=pms[:], lhsT=wm3c_sb[:64, 1:2], rhs=ms2[:64, 1, :],
                start=False, stop=True,
            )
            ms_sb = cp.tile([1, N], F32, name=f"ms_sb{rep}")
            nc.vector.tensor_copy(out=ms_sb[:], in_=pms[:])
            # broadcast ms over the 48 rows and add the causal mask
            pbc = pp.tile([R, N], F32, tag="big", name=f"pbc{rep}")
            nc.tensor.matmul(
                out=pbc[:], lhsT=one1[:], rhs=ms_sb[:], start=True, stop=True
            )
            nc.vector.tensor_tensor(
                out=mskms[:], in0=pbc[:], in1=maskb_sb[:], op=OP.add
            )

        # ---------- character head (independent of the pair grid; runs
        # before/under the main loop so its PSUM frees early) ----------
        cev = cp.tile([R, 1], F32, name=f"cev{rep}")
        with (
            tc.tile_pool(name=f"ch_sb{rep}", bufs=1) as chp,
            tc.tile_pool(name=f"ch_ps{rep}", bufs=2, space="PSUM") as chps,
        ):
            c1 = chp.tile([128, 3, R], BF16, name=f"c1{rep}")
            for co in range(3):
                pc = chps.tile([128, R], F32, tag="pc", name=f"pc{rep}_{co}")
                for ci in range(HC):
                    nc.tensor.matmul(
                        out=pc[:],
                        lhsT=wc1T_sb[:, ci, co * 128 : (co + 1) * 128],
                        rhs=repsTl_sb[:, ci, :],
                        start=(ci == 0),
                        stop=(ci == HC - 1),
                    )
                nc.scalar.activation(
                    out=c1[:, co, :], in_=pc[:], func=AF.Relu,
                    bias=bc1c_sb[:, co : co + 1],
                )
            plg = chps.tile([R, 18], F32, tag="lg", name=f"plg{rep}")
            for co in range(3):
                nc.tensor.matmul(
                    out=plg[:], lhsT=c1[:, co, :], rhs=wc2T_sb[:, co, :],
                    start=(co == 0), stop=False,
                )
            nc.tensor.matmul(
                out=plg[:], lhsT=one1[:], rhs=bc2r_sb[:], start=False, stop=True
            )
            cm = chp.tile([R, 1], F32, name=f"cm{rep}")
            nc.vector.tensor_reduce(
                out=cm[:], in_=plg[:], axis=mybir.AxisListType.X, op=OP.max
            )
            ncm = chp.tile([R, 1], F32, name=f"ncm{rep}")
            nc.vector.tensor_scalar_mul(ncm[:], cm[:], -1.0)
            cexp = chp.tile([R, 18], F32, name=f"cexp{rep}")
            cz = chp.tile([R, 1], F32, name=f"cz{rep}")
            nc.scalar.activation(
                out=cexp[:], in_=plg[:], func=AF.Exp, bias=ncm[:, 0:1],
                accum_out=cz[:],
            )
            cscr = chp.tile([R, 18], F32, name=f"cscr{rep}")
            nc.vector.tensor_tensor(
                out=cscr[:], in0=plg[:], in1=oneh_sb[:], op=OP.mult
            )
            sl = chp.tile([R, 1], F32, name=f"sl{rep}")
            nc.vector.tensor_reduce(
                out=sl[:], in_=cscr[:], axis=mybir.AxisListType.X, op=OP.add
            )
            lcz = chp.tile([R, 1], F32, name=f"lcz{rep}")
            nc.scalar.activation(out=lcz[:], in_=cz[:], func=AF.Ln)
            nc.vector.tensor_tensor(
                out=cev[:], in0=lcz[:], in1=cm[:], op=OP.add
            )
            nc.vector.tensor_tensor(
                out=cev[:], in0=cev[:], in1=sl[:], op=OP.subtract
            )

        # ---------- main loop: 48 rows of the pair grid ----------
        with (
            tc.tile_pool(name=f"lp_sb{rep}", bufs=2) as lsb,
            tc.tile_pool(name=f"lp_ps{rep}", bufs=2, space="PSUM") as lps,
            tc.tile_pool(name=f"sr_ps{rep}", bufs=2, space="PSUM") as sps,
        ):
            for r in range(R):
                # slot r holds global row i = 8r+1+d on core d (row 0 on
                # core 7's last slot); columns j < i fit in 8(r+1) exactly.
                cols = min(8 * (r + 1), N)
                h1 = lsb.tile(
                    [128, HC, N], F8, tag="h1", name=f"h1_{rep}_{r}",
                    bufs=opts["h1_bufs"],
                )
                for c in range(HC):
                    eng = nc.gpsimd if c >= HC - opts["h1_gp"] else nc.vector
                    eng.tensor_scalar(
                        out=h1[:, c, :cols],
                        in0=at_sb[:, c, :cols],
                        scalar1=bb_sb[:, c, r : r + 1],
                        scalar2=0.0,
                        op0=OP.add,
                        op1=OP.max,
                    )
                h2t = lsb.tile(
                    [128, 3, N], F8, tag="h2t", name=f"h2t_{rep}_{r}",
                    bufs=opts["h2t_bufs"],
                )
                ph = lps.tile(
                    [128, 3, 512], F32, tag="ph", name=f"ph{rep}_{r}", bufs=2
                )
                for hb in range(3):
                    for cc in range(0, HC, 2):
                        nc.tensor.matmul(
                            out=ph[:, hb, :cols],
                            lhsT=w28_sb[:, cc : cc + 2, hb * 128 : (hb + 1) * 128],
                            rhs=h1[:, cc : cc + 2, :cols],
                            start=(cc == 0),
                            stop=(cc == HC - 2),
                            perf_mode=DR,
                        )
                if fuse_relu:
                    # one 3D-AP relu over all 3 output blocks (b_pair2 == 0)
                    nc.scalar.activation(
                        out=h2t[:, :, :cols],
                        in_=ph[:, :, :cols],
                        func=AF.Relu,
                        scale=1.0 / S2,
                    )
                else:
                    for hb in range(3):
                        nc.scalar.activation(
                            out=h2t[:, hb, :cols],
                            in_=ph[:, hb, :cols],
                            func=AF.Relu,
                            bias=b2c_sb[:, hb : hb + 1],
                            scale=1.0 / S2,
                        )
                sr = sps.tile([1, N], F32, tag="srow", name=f"sr{rep}_{r}")
                nc.tensor.matmul(
                    out=sr[:, :cols],
                    lhsT=w38_sb[:, 0:2, 0:1],
                    rhs=h2t[:, 0:2, :cols],
                    start=True,
                    stop=False,
                    perf_mode=DR,
                )
                nc.tensor.matmul(
                    out=sr[:, :cols],
                    lhsT=w38_sb[:, 2, 0:1],
                    rhs=h2t[:, 2, :cols],
                    start=False,
                    stop=True,
                )
                # scores live on partition 0; engines can't shift
                # partitions, so stage flat and DMA-reshape later. The
                # 1/S3 fp8 descale folds into the copy; rotate engines
                # to split the load.
                # GPSIMD cannot read PSUM on HW; split DVE/ACT.
                dst = sblkf[:, r, :cols]
                cm = opts["copy_mode"]
                use_dve = (r % 2 == 0) if cm == "alt" else (cm == "dve")
                if use_dve:
                    nc.vector.tensor_scalar_mul(dst, sr[:, :cols], 1.0 / S3)
                else:
                    nc.scalar.mul(out=dst, in_=sr[:, :cols], mul=1.0 / S3)

        # ---------- epilogue: masked row-softmax loss + char CE ----------
        with (
            tc.tile_pool(name=f"ep_sb{rep}", bufs=1) as ep,
            tc.tile_pool(name=f"ep_ps{rep}", bufs=2, space="PSUM") as eps,
        ):
            # reshape valid score prefixes to [R, N] (per-slot DMAs ride
            # the idle queue); sblk was pre-zeroed so masked tails stay 0.
            for r in range(R):
                cols = min(8 * (r + 1), N)
                nc.sync.dma_start(
                    out=sblk[r : r + 1, :cols], in_=sblkf[:, r, :cols]
                )
            # row-softmax in 16-row groups so each group's chain starts as
            # soon as its score rows land (overlaps the main loop's tail).
            # Full-height tiles + sliced ops keep partitions lane-aligned.
            x = ep.tile([R, N], F32, name=f"x{rep}")
            rm = ep.tile([R, 1], F32, name=f"rm{rep}")
            nrm = ep.tile([R, 1], F32, name=f"nrm{rep}")
            pexp = ep.tile([R, N], F32, name=f"pexp{rep}")
            z = ep.tile([R, 1], F32, name=f"z{rep}")
            escr = ep.tile([R, N], F32, name=f"escr{rep}")
            e = ep.tile([R, 1], F32, name=f"e{rep}")
            lz = ep.tile([R, 1], F32, name=f"lz{rep}")
            le = ep.tile([R, 1], F32, name=f"le{rep}")
            tnll = ep.tile([R, 1], F32, name=f"tnll{rep}")
            for g0, g1 in ((0, 32), (32, 48)):  # engine partition base
                gs = slice(g0, g1)              # must be 32-aligned
                nc.vector.tensor_tensor(
                    out=x[gs, :], in0=sblk[gs, :], in1=mskms[gs, :], op=OP.add
                )
                nc.vector.tensor_reduce(
                    out=rm[gs, :], in_=x[gs, :], axis=mybir.AxisListType.X,
                    op=OP.max,
                )
                nc.vector.tensor_scalar_mul(nrm[gs, :], rm[gs, :], -1.0)
                nc.scalar.activation(
                    out=pexp[gs, :], in_=x[gs, :], func=AF.Exp,
                    bias=nrm[gs, 0:1], accum_out=z[gs, :],
                )
                nc.vector.tensor_tensor(
                    out=escr[gs, :], in0=pexp[gs, :], in1=multb_sb[gs, :],
                    op=OP.mult,
                )
                nc.vector.tensor_reduce(
                    out=e[gs, :], in_=escr[gs, :], axis=mybir.AxisListType.X,
                    op=OP.add,
                )
                nc.scalar.activation(out=lz[gs, :], in_=z[gs, :], func=AF.Ln)
                nc.scalar.activation(out=le[gs, :], in_=e[gs, :], func=AF.Ln)
                nc.vector.tensor_tensor(
                    out=tnll[gs, :], in0=lz[gs, :], in1=le[gs, :],
                    op=OP.subtract,
                )
            pl = eps.tile([1, 1], F32, tag="loss", name=f"pl{rep}", bufs=1)
            nc.tensor.matmul(
                out=pl[:], lhsT=tnll[:, 0:1], rhs=wnll_sb[:], start=True,
                stop=False,
            )
            nc.tensor.matmul(
                out=pl[:], lhsT=cev[:, 0:1], rhs=wch_sb[:], start=False,
                stop=True,
            )
            lout = ep.tile([1, 1], F32, name=f"lout{rep}")
            nc.vector.tensor_copy(out=lout[:], in_=pl[:])
            nc.sync.dma_start(out=io["loss"].ap(), in_=lout[:])


def _chunk_cols(w):
    """[K, O] -> [128, K//128, O]  (partition-chunked contraction dim)."""
    k, o = w.shape
    return np.ascontiguousarray(w.reshape(k // 128, 128, o).transpose(1, 0, 2))


def _chunk_vec(v, ncol):
    """[C] -> [128, ncol] column-chunks (zero padded)."""
    out = np.zeros((128, ncol), np.float32)
    for c in range(ncol):
        seg = v[c * 128 : (c + 1) * 128]
        out[: len(seg), c] = seg
    return out


def _core_rows(d):
    """Slot->global-row map for core d (16 rows of each chunk class)."""
    c1 = list(range(1, 129))
    c2 = list(range(129, 257))
    c3 = list(range(257, 384)) + [0]
    return c1[d::NC_] + c2[d::NC_] + c3[d::NC_]


def _prep_in_maps(inputs):
    bf = ml_dtypes.bfloat16
    f8 = ml_dtypes.float8_e4m3

    seq = np.asarray(inputs["sequence_output"], np.float32)
    spk = np.asarray(inputs["speaker_emb"], np.float32)
    dummy = np.asarray(inputs["dummy_emb"], np.float32)
    seg = np.asarray(inputs["mentions_seg"]).astype(np.int64)
    mstart = np.asarray(inputs["mention_start"]).astype(np.int64)
    mend = np.asarray(inputs["mention_end"]).astype(np.int64)
    sid = np.asarray(inputs["speaker_ids"]).astype(np.int64)[seg, mstart]
    reps = seq[seg, mstart] + seq[seg, mend] + spk[sid]
    all_reps = np.concatenate([dummy, reps], axis=0)          # [N, H]

    def chunkT(a):
        # [n, H] -> [128, HC, n] transposed layout
        n = a.shape[0]
        return np.ascontiguousarray(
            a.T.reshape(HC, 128, n).transpose(1, 0, 2)
        )

    reps8 = chunkT(all_reps).astype(f8)

    W_pair1 = np.asarray(inputs["W_pair1"], np.float32)
    wa8 = _chunk_cols(np.ascontiguousarray(W_pair1[:, :H].T) * S2).astype(f8)
    wb8 = _chunk_cols(np.ascontiguousarray(W_pair1[:, H:].T) * S2).astype(f8)
    w28 = _chunk_cols(
        np.ascontiguousarray(np.asarray(inputs["W_pair2"], np.float32).T) * S2
    ).astype(f8)
    w38 = np.zeros((128, 3, 16), np.float32)
    w38[:, :, 0] = _chunk_vec(
        np.asarray(inputs["W_pair3"], np.float32)[0] * S3, 3
    )
    w38 = w38.astype(f8)
    b1c = _chunk_vec(np.asarray(inputs["b_pair1"], np.float32), HC)
    b2c = _chunk_vec(np.asarray(inputs["b_pair2"], np.float32), 3)
    wm18 = _chunk_cols(
        np.ascontiguousarray(np.asarray(inputs["W_m1"], np.float32).T) * S2
    ).astype(f8)
    bm1c = _chunk_vec(np.asarray(inputs["b_m1"], np.float32), 3)
    wm2T = _chunk_cols(
        np.ascontiguousarray(np.asarray(inputs["W_m2"], np.float32).T)
    ).astype(bf)
    bm2c = _chunk_vec(np.asarray(inputs["b_m2"], np.float32), 2)
    wm3c = _chunk_vec(np.asarray(inputs["W_m3"], np.float32)[0], 2).astype(bf)
    wc1T = _chunk_cols(
        np.ascontiguousarray(np.asarray(inputs["W_c1"], np.float32).T)
    ).astype(bf)
    bc1c = _chunk_vec(np.asarray(inputs["b_c1"], np.float32), 3)
    wc2T = _chunk_cols(
        np.ascontiguousarray(np.asarray(inputs["W_c2"], np.float32).T)
    ).astype(bf)
    bc2r = np.asarray(inputs["b_c2"], np.float32).reshape(1, 18)

    link_first = np.asarray(inputs["link_first"]).astype(np.int64)
    link_second = np.asarray(inputs["link_second"]).astype(np.int64)
    label = np.asarray(inputs["character_label"]).astype(np.int64)

    mult = np.zeros((N, N), np.float32)
    np.add.at(mult, (link_second, link_first), 1.0)
    has_link = mult.sum(axis=1) > 0
    wnll_full = ((np.arange(N) >= 1) & has_link).astype(np.float32)
    mult[~has_link, 0] = 1.0  # keep log(E) finite; weight is 0 there

    mask_full = np.where(
        np.arange(N)[None, :] >= np.arange(N)[:, None], np.float32(NEG), 0.0
    ).astype(np.float32)

    oneh_full = np.zeros((N, 18), np.float32)
    wch_full = np.zeros(N, np.float32)
    oneh_full[np.arange(1, N), label] = 1.0
    wch_full[1:] = 1.0

    shared = dict(
        reps8=reps8,
        wa8=wa8, wb8=wb8, w28=w28, w38=w38, b1c=b1c, b2c=b2c,
        wm18=wm18, bm1c=bm1c, wm2T=wm2T, bm2c=bm2c, wm3c=wm3c,
        wc1T=wc1T, bc1c=bc1c, wc2T=wc2T, bc2r=bc2r,
        zrow=np.zeros((R, N), np.float32),
    )
    in_maps = []
    for d in range(NC_):
        rows = _core_rows(d)
        m = dict(shared)
        m["repsTl"] = chunkT(all_reps[rows]).astype(bf)
        m["repsTl8"] = chunkT(all_reps[rows]).astype(f8)
        m["maskb"] = np.ascontiguousarray(mask_full[rows])
        m["multb"] = np.ascontiguousarray(mult[rows])
        m["wnll"] = np.ascontiguousarray(wnll_full[rows]).reshape(R, 1)
        m["oneh"] = np.ascontiguousarray(oneh_full[rows])
        m["wch"] = np.ascontiguousarray(wch_full[rows]).reshape(R, 1)
        in_maps.append(m)
    return in_maps


def kernel(**inputs):
    global LAST_RESULT
    in_maps = _prep_in_maps(inputs)

    # the fused h2 relu drops the (per-spec zero) b_pair2 bias
    fuse = not np.any(np.asarray(inputs["b_pair2"], np.float32))
    key = ("nc", fuse)
    if key not in _CACHE:
        _CACHE[key] = _build_program(fuse_relu=fuse)
    nc = _CACHE[key]

    res = run_bass_kernel_spmd(nc, in_maps, core_ids=list(range(NC_)))
    LAST_RESULT = res
    total = np.float32(0.0)
    for d in range(NC_):
        total += np.float32(res.results[d]["loss"][0, 0])
    return np.asarray(total, dtype=np.float32)


if __name__ == "__main__":
    import reference

    inputs = {k: np.asarray(v) for k, v in reference.setup_inputs().items()}
    out = kernel(**inputs)
    print("kernel out:", out)


# revision 56
# speedup vs baseline: 1.1578x; 1.1578x over previous
"""Trainium2 Bass kernel for nn_JointLearningModel (coref-style joint model).

Sharding: the 384x384 pair grid is split by rows across 8 NeuronCores.
Mention representations are computed on the host (pure gathers) and
uploaded replicated in transposed [H, N] layout; params replicated; the
scalar loss is computed per-core over its row block (+ its slice of the
character CE) and summed on the host.

Key optimizations over the naive version:
- The causal mask means row i only needs pair columns j < i. Rows are
  dealt to cores by column-chunk class (1, 2 or 3 x128 chunks) so all
  cores run the identical SPMD instruction stream but skip ~1/3 of the
  pair-MLP work.
- The dominant W_pair2 matmul (and the W_pair3 reduction) run in fp8
  (e4m3, weights pre-scaled by 16) with DoubleRow perf mode: 2x PE
  throughput, contraction 256 per instruction.
- Score rows move PSUM->SBUF via DMA instead of vector/scalar copies.
"""

import numpy as np
import ml_dtypes

import concourse.bass as bass
import concourse.mybir as mybir
import concourse.tile as tile
from concourse import bacc
from concourse.bass_utils import run_bass_kernel_spmd

F32 = mybir.dt.float32
BF16 = mybir.dt.bfloat16
F8 = mybir.dt.float8e4
I32 = mybir.dt.int32
AF = mybir.ActivationFunctionType
OP = mybir.AluOpType
DR = mybir.MatmulPerfMode.DoubleRow

B, L, H, M = 8, 512, 768, 383
N = M + 1          # 384 rows/cols of the pair grid
NC_ = 8            # cores
R = N // NC_       # 48 rows per core
HC = H // 128      # 6 k-chunks of the hidden dim
NEG = -10000.0
S2 = 16.0          # fp8 pre-scale on W_pair2
S3 = 16.0          # fp8 pre-scale on W_pair3

_CACHE = {}
LAST_RESULT = None


def _build_program(
    reps=1, fuse_relu=True, h1_gp=0, h1_bufs=8, h2t_bufs=4, copy_mode="act"
):
    nc = bacc.Bacc(
        "TRN2", target_bir_lowering=False, debug=False, enable_asserts=False
    )

    def din(name, shape, dt):
        return nc.dram_tensor(name, list(shape), dt, kind="ExternalInput")

    # mention representations (host-gathered), transposed layouts
    reps8 = din("reps8", [128, HC, N], F8)      # reps8[p,c,j] = reps[j, 128c+p]
    repsTl = din("repsTl", [128, HC, R], BF16)  # local rows, slot order
    repsTl8 = din("repsTl8", [128, HC, R], F8)
    # pair MLP weights (fp8, pre-scaled by S2)
    wa8 = din("wa8", [128, HC, H], F8)          # wa8[p,ci,o] = Wa[o, 128ci+p]*S2
    wb8 = din("wb8", [128, HC, H], F8)
    w28 = din("w28", [128, HC, H // 2], F8)     # W2.T * S2, fp8
    # inner dim padded to 16 so the DoubleRow k-pair stride is 16B-aligned
    w38 = din("w38", [128, 3, 16], F8)          # W3 * S3 in [:, :, 0], fp8
    b1c = din("b1c", [128, HC], F32)
    b2c = din("b2c", [128, 3], F32)
    # mention-score MLP
    wm18 = din("wm18", [128, HC, H // 2], F8)   # W_m1.T * S2, fp8
    bm1c = din("bm1c", [128, 3], F32)
    wm2T = din("wm2T", [128, 3, H // 4], BF16)
    bm2c = din("bm2c", [128, 2], F32)
    wm3c = din("wm3c", [128, 2], BF16)
    # character head
    wc1T = din("wc1T", [128, HC, H // 2], BF16)
    bc1c = din("bc1c", [128, 3], F32)
    wc2T = din("wc2T", [128, 3, 18], BF16)
    bc2r = din("bc2r", [1, 18], F32)
    # per-core loss plumbing (rows in slot order)
    maskb = din("maskb", [R, N], F32)
    multb = din("multb", [R, N], F32)
    wnll = din("wnll", [R, 1], F32)
    oneh = din("oneh", [R, 18], F32)
    wch = din("wch", [R, 1], F32)
    zrow = din("zrow", [R, N], F32)             # zeros, pre-clears sblk

    loss = nc.dram_tensor("loss", [1, 1], F32, kind="ExternalOutput")

    opts = dict(fuse_relu=fuse_relu, h1_gp=h1_gp, h1_bufs=h1_bufs,
                h2t_bufs=h2t_bufs, copy_mode=copy_mode)
    with tile.TileContext(nc) as tc:
        for rep in range(reps):
            _build_rep(nc, tc, rep, opts, dict(
                reps8=reps8, repsTl=repsTl, repsTl8=repsTl8, wa8=wa8,
                wb8=wb8, w28=w28, w38=w38, b1c=b1c, b2c=b2c, wm18=wm18,
                bm1c=bm1c, wm2T=wm2T, bm2c=bm2c, wm3c=wm3c, wc1T=wc1T,
                bc1c=bc1c, wc2T=wc2T, bc2r=bc2r, maskb=maskb,
                multb=multb, wnll=wnll, oneh=oneh, wch=wch, zrow=zrow,
                loss=loss,
            ))

    nc.compile()
    return nc


def _build_rep(nc, tc, rep, opts, io):
    fuse_relu = opts["fuse_relu"]
    with tc.tile_pool(name=f"const{rep}", bufs=1) as cp:
        def load(name, h, q=None):
            t = cp.tile(list(h.shape), h.dtype, name=f"{name}{rep}")
            (q or nc.sync).dma_start(out=t[:], in_=h.ap())
            return t

        # split the loads over both HWDGE queues (SP + ACT) so the A.T
        # and Bb input chains start in parallel
        reps8_sb = load("reps8_sb", io["reps8"])
        wa8_sb = load("wa8_sb", io["wa8"])
        wb8_sb = load("wb8_sb", io["wb8"], q=nc.scalar)
        repsTl_sb = load("repsTl_sb", io["repsTl"])
        repsTl8_sb = load("repsTl8_sb", io["repsTl8"], q=nc.scalar)
        w28_sb = load("w28_sb", io["w28"])
        w38_sb = load("w38_sb", io["w38"])
        b1c_sb = load("b1c_sb", io["b1c"], q=nc.scalar)
        b2c_sb = None if fuse_relu else load("b2c_sb", io["b2c"])
        wm18_sb = load("wm18_sb", io["wm18"])
        bm1c_sb = load("bm1c_sb", io["bm1c"])
        wm2T_sb = load("wm2T_sb", io["wm2T"])
        bm2c_sb = load("bm2c_sb", io["bm2c"])
        wm3c_sb = load("wm3c_sb", io["wm3c"])
        wc1T_sb = load("wc1T_sb", io["wc1T"], q=nc.scalar)
        bc1c_sb = load("bc1c_sb", io["bc1c"], q=nc.scalar)
        wc2T_sb = load("wc2T_sb", io["wc2T"], q=nc.scalar)
        bc2r_sb = load("bc2r_sb", io["bc2r"], q=nc.scalar)
        maskb_sb = load("maskb_sb", io["maskb"], q=nc.scalar)
        multb_sb = load("multb_sb", io["multb"], q=nc.scalar)
        wnll_sb = load("wnll_sb", io["wnll"], q=nc.scalar)
        oneh_sb = load("oneh_sb", io["oneh"], q=nc.scalar)
        wch_sb = load("wch_sb", io["wch"], q=nc.scalar)

        one1 = cp.tile([1, R], F32, name=f"one1{rep}")
        nc.vector.memset(one1[:], 1.0)

        # outputs of the preamble, used by the main loop / epilogue
        at_sb = cp.tile([128, HC, N], BF16, name=f"at_sb{rep}")   # A.T
        bb_sb = cp.tile([128, HC, R], F32, name=f"bb_sb{rep}")    # Bm.T + b1
        mskms = cp.tile([R, N], F32, name=f"mskms{rep}")          # mask+ms[j]
        sblkf = cp.tile([1, R, N], F32, name=f"sblkf{rep}")       # scores flat
        # Destination of the reshape; pre-zeroed (fast: spreads over 48
        # partitions) so rows' uncomputed masked tails read as 0.
        sblk = cp.tile([R, N], F32, name=f"sblk{rep}")
        nc.sync.dma_start(out=sblk[:], in_=io["zrow"].ap())

        # ---------- preamble matmuls: A.T, Bb, ms, mask+ms ----------
        with tc.tile_pool(name=f"pre_ps{rep}", bufs=2, space="PSUM") as pp:
            for co in range(HC):
                pa = pp.tile([128, N], F32, tag="big", name=f"pa{rep}_{co}")
                for ci in range(0, HC, 2):
                    nc.tensor.matmul(
                        out=pa[:],
                        lhsT=wa8_sb[:, ci : ci + 2, co * 128 : (co + 1) * 128],
                        rhs=reps8_sb[:, ci : ci + 2, :],
                        start=(ci == 0),
                        stop=(ci == HC - 2),
                        perf_mode=DR,
                    )
                nc.scalar.mul(out=at_sb[:, co, :], in_=pa[:], mul=1.0 / S2)
            for co in range(HC):
                pb = pp.tile([128, R], F32, tag="small", name=f"pb{rep}_{co}")
                for ci in range(HC):
                    nc.tensor.matmul(
                        out=pb[:],
                        lhsT=wb8_sb[:, ci, co * 128 : (co + 1) * 128],
                        rhs=repsTl8_sb[:, ci, :],
                        start=(ci == 0),
                        stop=(ci == HC - 1),
                    )
                nc.vector.tensor_scalar(
                    out=bb_sb[:, co, :],
                    in0=pb[:],
                    scalar1=1.0 / S2,
                    scalar2=b1c_sb[:, co : co + 1],
                    op0=OP.mult,
                    op1=OP.add,
                )
            # mention score MLP (768 -> 384 -> 192 -> 1)
            ms1 = cp.tile([128, 3, N], BF16, name=f"ms1{rep}")
            for co in range(3):
                pm = pp.tile([128, N], F32, tag="big", name=f"pm{rep}_{co}")
                for ci in range(0, HC, 2):
                    nc.tensor.matmul(
                        out=pm[:],
                        lhsT=wm18_sb[:, ci : ci + 2, co * 128 : (co + 1) * 128],
                        rhs=reps8_sb[:, ci : ci + 2, :],
                        start=(ci == 0),
                        stop=(ci == HC - 2),
                        perf_mode=DR,
                    )
                nc.scalar.activation(
                    out=ms1[:, co, :],
                    in_=pm[:],
                    func=AF.Relu,
                    bias=bm1c_sb[:, co : co + 1],
                    scale=1.0 / S2,
                )
            ms2 = cp.tile([128, 2, N], BF16, name=f"ms2{rep}")
            for co, sz in enumerate((128, 64)):
                pm2 = pp.tile([128, N], F32, tag="big", name=f"pm2{rep}_{co}")
                for ci in range(3):
                    nc.tensor.matmul(
                        out=pm2[:sz, :],
                        lhsT=wm2T_sb[:, ci, co * 128 : co * 128 + sz],
                        rhs=ms1[:, ci, :],
                        start=(ci == 0),
                        stop=(ci == 2),
                    )
                nc.scalar.activation(
                    out=ms2[:sz, co, :],
                    in_=pm2[:sz, :],
                    func=AF.Relu,
                    bias=bm2c_sb[:sz, co : co + 1],
                )
            pms = pp.tile([1, N], F32, tag="small", name=f"pms{rep}")
            nc.tensor.matmul(
                out=pms[:], lhsT=wm3c_sb[:, 0:1], rhs=ms2[:, 0, :],
                start=True, stop=False,
            )
            nc.tensor.matmul(
                out=pms[:], lhsT=wm3c_sb[:64, 1:2], rhs=ms2[:64, 1, :],
                start=False, stop=True,
            )
            ms_sb = cp.tile([1, N], F32, name=f"ms_sb{rep}")
            nc.vector.tensor_copy(out=ms_sb[:], in_=pms[:])
            # broadcast ms over the 48 rows and add the causal mask
            pbc = pp.tile([R, N], F32, tag="big", name=f"pbc{rep}")
            nc.tensor.matmul(
                out=pbc[:], lhsT=one1[:], rhs=ms_sb[:], start=True, stop=True
            )
            nc.vector.tensor_tensor(
                out=mskms[:], in0=pbc[:], in1=maskb_sb[:], op=OP.add
            )

        # ---------- character head (independent of the pair grid; runs
        # before/under the main loop so its PSUM frees early) ----------
        cev = cp.tile([R, 1], F32, name=f"cev{rep}")
        with (
            tc.tile_pool(name=f"ch_sb{rep}", bufs=1) as chp,
            tc.tile_pool(name=f"ch_ps{rep}", bufs=2, space="PSUM") as chps,
        ):
            c1 = chp.tile([128, 3, R], BF16, name=f"c1{rep}")
            for co in range(3):
                pc = chps.tile([128, R], F32, tag="pc", name=f"pc{rep}_{co}")
                for ci in range(HC):
                    nc.tensor.matmul(
                        out=pc[:],
                        lhsT=wc1T_sb[:, ci, co * 128 : (co + 1) * 128],
                        rhs=repsTl_sb[:, ci, :],
                        start=(ci == 0),
                        stop=(ci == HC - 1),
                    )
                nc.scalar.activation(
                    out=c1[:, co, :], in_=pc[:], func=AF.Relu,
                    bias=bc1c_sb[:, co : co + 1],
                )
            plg = chps.tile([R, 18], F32, tag="lg", name=f"plg{rep}")
            for co in range(3):
                nc.tensor.matmul(
                    out=plg[:], lhsT=c1[:, co, :], rhs=wc2T_sb[:, co, :],
                    start=(co == 0), stop=False,
                )
            nc.tensor.matmul(
                out=plg[:], lhsT=one1[:], rhs=bc2r_sb[:], start=False, stop=True
            )
            cm = chp.tile([R, 1], F32, name=f"cm{rep}")
            nc.vector.tensor_reduce(
                out=cm[:], in_=plg[:], axis=mybir.AxisListType.X, op=OP.max
            )
            ncm = chp.tile([R, 1], F32, name=f"ncm{rep}")
            nc.vector.tensor_scalar_mul(ncm[:], cm[:], -1.0)
            cexp = chp.tile([R, 18], F32, name=f"cexp{rep}")
            cz = chp.tile([R, 1], F32, name=f"cz{rep}")
            nc.scalar.activation(
                out=cexp[:], in_=plg[:], func=AF.Exp, bias=ncm[:, 0:1],
                accum_out=cz[:],
            )
            cscr = chp.tile([R, 18], F32, name=f"cscr{rep}")
            nc.vector.tensor_tensor(
                out=cscr[:], in0=plg[:], in1=oneh_sb[:], op=OP.mult
            )
            sl = chp.tile([R, 1], F32, name=f"sl{rep}")
            nc.vector.tensor_reduce(
                out=sl[:], in_=cscr[:], axis=mybir.AxisListType.X, op=OP.add
            )
            lcz = chp.tile([R, 1], F32, name=f"lcz{rep}")
            nc.scalar.activation(out=lcz[:], in_=cz[:], func=AF.Ln)
            nc.vector.tensor_tensor(
                out=cev[:], in0=lcz[:], in1=cm[:], op=OP.add
            )
            nc.vector.tensor_tensor(
                out=cev[:], in0=cev[:], in1=sl[:], op=OP.subtract
            )

        # ---------- main loop: 48 rows of the pair grid ----------
        with (
            tc.tile_pool(name=f"lp_sb{rep}", bufs=2) as lsb,
            tc.tile_pool(name=f"lp_ps{rep}", bufs=2, space="PSUM") as lps,
            tc.tile_pool(name=f"sr_ps{rep}", bufs=2, space="PSUM") as sps,
        ):
            for r in range(R):
                # slot r holds global row i = 8r+1+d on core d (row 0 on
                # core 7's last slot); columns j < i fit in 8(r+1) exactly.
                cols = min(8 * (r + 1), N)
                h1 = lsb.tile(
                    [128, HC, N], F8, tag="h1", name=f"h1_{rep}_{r}",
                    bufs=opts["h1_bufs"],
                )
                for c in range(HC):
                    eng = nc.gpsimd if c >= HC - opts["h1_gp"] else nc.vector
                    eng.tensor_scalar(
                        out=h1[:, c, :cols],
                        in0=at_sb[:, c, :cols],
                        scalar1=bb_sb[:, c, r : r + 1],
                        scalar2=0.0,
                        op0=OP.add,
                        op1=OP.max,
                    )
                h2t = lsb.tile(
                    [128, 3, N], F8, tag="h2t", name=f"h2t_{rep}_{r}",
                    bufs=opts["h2t_bufs"],
                )
                ph = lps.tile(
                    [128, 3, 512], F32, tag="ph", name=f"ph{rep}_{r}", bufs=2
                )
                for hb in range(3):
                    for cc in range(0, HC, 2):
                        nc.tensor.matmul(
                            out=ph[:, hb, :cols],
                            lhsT=w28_sb[:, cc : cc + 2, hb * 128 : (hb + 1) * 128],
                            rhs=h1[:, cc : cc + 2, :cols],
                            start=(cc == 0),
                            stop=(cc == HC - 2),
                            perf_mode=DR,
                        )
                if fuse_relu:
                    # one 3D-AP relu over all 3 output blocks (b_pair2 == 0)
                    nc.scalar.activation(
                        out=h2t[:, :, :cols],
                        in_=ph[:, :, :cols],
                        func=AF.Relu,
                        scale=1.0 / S2,
                    )
                else:
                    for hb in range(3):
                        nc.scalar.activation(
                            out=h2t[:, hb, :cols],
                            in_=ph[:, hb, :cols],
                            func=AF.Relu,
                            bias=b2c_sb[:, hb : hb + 1],
                            scale=1.0 / S2,
                        )
                sr = sps.tile([1, N], F32, tag="srow", name=f"sr{rep}_{r}")
                nc.tensor.matmul(
                    out=sr[:, :cols],
                    lhsT=w38_sb[:, 0:2, 0:1],
                    rhs=h2t[:, 0:2, :cols],
                    start=True,
                    stop=False,
                    perf_mode=DR,
                )
                nc.tensor.matmul(
                    out=sr[:, :cols],
                    lhsT=w38_sb[:, 2, 0:1],
                    rhs=h2t[:, 2, :cols],
                    start=False,
                    stop=True,
                )
                # scores live on partition 0; engines can't shift
                # partitions, so stage flat and DMA-reshape later. The
                # 1/S3 fp8 descale folds into the copy; rotate engines
                # to split the load.
                # GPSIMD cannot read PSUM on HW; split DVE/ACT.
                dst = sblkf[:, r, :cols]
                cm = opts["copy_mode"]
                use_dve = (r % 2 == 0) if cm == "alt" else (cm == "dve")
                if use_dve:
                    nc.vector.tensor_scalar_mul(dst, sr[:, :cols], 1.0 / S3)
                else:
                    nc.scalar.mul(out=dst, in_=sr[:, :cols], mul=1.0 / S3)

        # ---------- epilogue: masked row-softmax loss + char CE ----------
        with (
            tc.tile_pool(name=f"ep_sb{rep}", bufs=1) as ep,
            tc.tile_pool(name=f"ep_ps{rep}", bufs=2, space="PSUM") as eps,
        ):
            # reshape valid score prefixes to [R, N] (per-slot DMAs ride
            # the idle queue); sblk was pre-zeroed so masked tails stay 0.
            for r in range(R):
                cols = min(8 * (r + 1), N)
                nc.sync.dma_start(
                    out=sblk[r : r + 1, :cols], in_=sblkf[:, r, :cols]
                )
            # row-softmax in 16-row groups so each group's chain starts as
            # soon as its score rows land (overlaps the main loop's tail).
            # Full-height tiles + sliced ops keep partitions lane-aligned.
            x = ep.tile([R, N], F32, name=f"x{rep}")
            rm = ep.tile([R, 1], F32, name=f"rm{rep}")
            nrm = ep.tile([R, 1], F32, name=f"nrm{rep}")
            pexp = ep.tile([R, N], F32, name=f"pexp{rep}")
            z = ep.tile([R, 1], F32, name=f"z{rep}")
            escr = ep.tile([R, N], F32, name=f"escr{rep}")
            e = ep.tile([R, 1], F32, name=f"e{rep}")
            lz = ep.tile([R, 1], F32, name=f"lz{rep}")
            le = ep.tile([R, 1], F32, name=f"le{rep}")
            tnll = ep.tile([R, 1], F32, name=f"tnll{rep}")
            for g0, g1 in ((0, 32), (32, 48)):  # engine partition base
                gs = slice(g0, g1)              # must be 32-aligned
                nc.vector.tensor_tensor(
                    out=x[gs, :], in0=sblk[gs, :], in1=mskms[gs, :], op=OP.add
                )
                nc.vector.tensor_reduce(
                    out=rm[gs, :], in_=x[gs, :], axis=mybir.AxisListType.X,
                    op=OP.max,
                )
                nc.vector.tensor_scalar_mul(nrm[gs, :], rm[gs, :], -1.0)
                nc.scalar.activation(
                    out=pexp[gs, :], in_=x[gs, :], func=AF.Exp,
                    bias=nrm[gs, 0:1], accum_out=z[gs, :],
                )
                nc.vector.tensor_tensor(
                    out=escr[gs, :], in0=pexp[gs, :], in1=multb_sb[gs, :],
                    op=OP.mult,
                )
                nc.vector.tensor_reduce(
                    out=e[gs, :], in_=escr[gs, :], axis=mybir.AxisListType.X,
                    op=OP.add,
                )
                nc.scalar.activation(out=lz[gs, :], in_=z[gs, :], func=AF.Ln)
                nc.scalar.activation(out=le[gs, :], in_=e[gs, :], func=AF.Ln)
                nc.vector.tensor_tensor(
                    out=tnll[gs, :], in0=lz[gs, :], in1=le[gs, :],
                    op=OP.subtract,
                )
            pl = eps.tile([1, 1], F32, tag="loss", name=f"pl{rep}", bufs=1)
            nc.tensor.matmul(
                out=pl[:], lhsT=tnll[:, 0:1], rhs=wnll_sb[:], start=True,
                stop=False,
            )
            nc.tensor.matmul(
                out=pl[:], lhsT=cev[:, 0:1], rhs=wch_sb[:], start=False,
                stop=True,
            )
            lout = ep.tile([1, 1], F32, name=f"lout{rep}")
            nc.vector.tensor_copy(out=lout[:], in_=pl[:])
            nc.sync.dma_start(out=io["loss"].ap(), in_=lout[:])


def _chunk_cols(w):
    """[K, O] -> [128, K//128, O]  (partition-chunked contraction dim)."""
    k, o = w.shape
    return np.ascontiguousarray(w.reshape(k // 128, 128, o).transpose(1, 0, 2))


def _chunk_vec(v, ncol):
    """[C] -> [128, ncol] column-chunks (zero padded)."""
    out = np.zeros((128, ncol), np.float32)
    for c in range(ncol):
        seg = v[c * 128 : (c + 1) * 128]
        out[: len(seg), c] = seg
    return out


def _core_rows(d):
    """Slot->global-row map for core d (16 rows of each chunk class)."""
    c1 = list(range(1, 129))
    c2 = list(range(129, 257))
    c3 = list(range(257, 384)) + [0]
    return c1[d::NC_] + c2[d::NC_] + c3[d::NC_]


def _prep_in_maps(inputs):
    bf = ml_dtypes.bfloat16
    f8 = ml_dtypes.float8_e4m3

    seq = np.asarray(inputs["sequence_output"], np.float32)
    spk = np.asarray(inputs["speaker_emb"], np.float32)
    dummy = np.asarray(inputs["dummy_emb"], np.float32)
    seg = np.asarray(inputs["mentions_seg"]).astype(np.int64)
    mstart = np.asarray(inputs["mention_start"]).astype(np.int64)
    mend = np.asarray(inputs["mention_end"]).astype(np.int64)
    sid = np.asarray(inputs["speaker_ids"]).astype(np.int64)[seg, mstart]
    reps = seq[seg, mstart] + seq[seg, mend] + spk[sid]
    all_reps = np.concatenate([dummy, reps], axis=0)          # [N, H]

    def chunkT(a):
        # [n, H] -> [128, HC, n] transposed layout
        n = a.shape[0]
        return np.ascontiguousarray(
            a.T.reshape(HC, 128, n).transpose(1, 0, 2)
        )

    reps8 = chunkT(all_reps).astype(f8)

    W_pair1 = np.asarray(inputs["W_pair1"], np.float32)
    wa8 = _chunk_cols(np.ascontiguousarray(W_pair1[:, :H].T) * S2).astype(f8)
    wb8 = _chunk_cols(np.ascontiguousarray(W_pair1[:, H:].T) * S2).astype(f8)
    w28 = _chunk_cols(
        np.ascontiguousarray(np.asarray(inputs["W_pair2"], np.float32).T) * S2
    ).astype(f8)
    w38 = np.zeros((128, 3, 16), np.float32)
    w38[:, :, 0] = _chunk_vec(
        np.asarray(inputs["W_pair3"], np.float32)[0] * S3, 3
    )
    w38 = w38.astype(f8)
    b1c = _chunk_vec(np.asarray(inputs["b_pair1"], np.float32), HC)
    b2c = _chunk_vec(np.asarray(inputs["b_pair2"], np.float32), 3)
    wm18 = _chunk_cols(
        np.ascontiguousarray(np.asarray(inputs["W_m1"], np.float32).T) * S2
    ).astype(f8)
    bm1c = _chunk_vec(np.asarray(inputs["b_m1"], np.float32), 3)
    wm2T = _chunk_cols(
        np.ascontiguousarray(np.asarray(inputs["W_m2"], np.float32).T)
    ).astype(bf)
    bm2c = _chunk_vec(np.asarray(inputs["b_m2"], np.float32), 2)
    wm3c = _chunk_vec(np.asarray(inputs["W_m3"], np.float32)[0], 2).astype(bf)
    wc1T = _chunk_cols(
        np.ascontiguousarray(np.asarray(inputs["W_c1"], np.float32).T)
    ).astype(bf)
    bc1c = _chunk_vec(np.asarray(inputs["b_c1"], np.float32), 3)
    wc2T = _chunk_cols(
        np.ascontiguousarray(np.asarray(inputs["W_c2"], np.float32).T)
    ).astype(bf)
    bc2r = np.asarray(inputs["b_c2"], np.float32).reshape(1, 18)

    link_first = np.asarray(inputs["link_first"]).astype(np.int64)
    link_second = np.asarray(inputs["link_second"]).astype(np.int64)
    label = np.asarray(inputs["character_label"]).astype(np.int64)

    mult = np.zeros((N, N), np.float32)
    np.add.at(mult, (link_second, link_first), 1.0)
    has_link = mult.sum(axis=1) > 0
    wnll_full = ((np.arange(N) >= 1) & has_link).astype(np.float32)
    mult[~has_link, 0] = 1.0  # keep log(E) finite; weight is 0 there

    mask_full = np.where(
        np.arange(N)[None, :] >= np.arange(N)[:, None], np.float32(NEG), 0.0
    ).astype(np.float32)

    oneh_full = np.zeros((N, 18), np.float32)
    wch_full = np.zeros(N, np.float32)
    oneh_full[np.arange(1, N), label] = 1.0
    wch_full[1:] = 1.0

    shared = dict(
        reps8=reps8,
        wa8=wa8, wb8=wb8, w28=w28, w38=w38, b1c=b1c, b2c=b2c,
        wm18=wm18, bm1c=bm1c, wm2T=wm2T, bm2c=bm2c, wm3c=wm3c,
        wc1T=wc1T, bc1c=bc1c, wc2T=wc2T, bc2r=bc2r,
        zrow=np.zeros((R, N), np.float32),
    )
    in_maps = []
    for d in range(NC_):
        rows = _core_rows(d)
        m = dict(shared)
        m["repsTl"] = chunkT(all_reps[rows]).astype(bf)
        m["repsTl8"] = chunkT(all_reps[rows]).astype(f8)
        m["maskb"] = np.ascontiguousarray(mask_full[rows])
        m["multb"] = np.ascontiguousarray(mult[rows])
        m["wnll"] = np.ascontiguousarray(wnll_full[rows]).reshape(R, 1)
        m["oneh"] = np.ascontiguousarray(oneh_full[rows])
        m["wch"] = np.ascontiguousarray(wch_full[rows]).reshape(R, 1)
        in_maps.append(m)
    return in_maps


def kernel(**inputs):
    global LAST_RESULT
    in_maps = _prep_in_maps(inputs)

    # the fused h2 relu drops the (per-spec zero) b_pair2 bias
    fuse = not np.any(np.asarray(inputs["b_pair2"], np.float32))
    key = ("nc", fuse)
    if key not in _CACHE:
        _CACHE[key] = _build_program(fuse_relu=fuse)
    nc = _CACHE[key]

    res = run_bass_kernel_spmd(nc, in_maps, core_ids=list(range(NC_)))
    LAST_RESULT = res
    total = np.float32(0.0)
    for d in range(NC_):
        total += np.float32(res.results[d]["loss"][0, 0])
    return np.asarray(total, dtype=np.float32)


if __name__ == "__main__":
    import reference

    inputs = {k: np.asarray(v) for k, v in reference.setup_inputs().items()}
    out = kernel(**inputs)
    print("kernel out:", out)
